# revision 26
# baseline (speedup 1.0000x reference)
"""Trainium2 Bass kernel for nn_PerClassGating (moe_routing).

Computes, for inputs features[B,F], Ws[F,H], bs[H], W1[C,H,K], b1[C,K],
W2[C,K,E], b2[C,E] (B=256, F=2048, H=512, K=H/2=256, C=512, E=8):

    shared      = relu(features @ Ws + bs)                 # [B, H]
    h           = relu(einsum('bh,chk->bck', shared, W1) + b1)
    gate_logits = einsum('bck,cke->bce', h, W2) + b2       # [B, C, E]
    gate_weights = softmax(gate_logits, axis=-1)

Sharding: the class dim C is split across 8 NeuronCores (64 classes per
core); features and the shared transform are replicated. No collectives —
each core produces a disjoint [B, 64, E] slab of both outputs.

All matmul operands are bf16, cast on the HOST (tolerance is 2e-2;
bf16 keeps rel-err ~4e-3). This halves the dominant W1 HBM traffic
vs fp32/fp32r (16 MiB/core instead of 32), lets every load use the
fast HWDGE DMA path (no SWDGE cast), and enables the PE's automatic
Fast-Weight-Load (FWL) for 128-col bf16 stationaries, which halves
LDWEIGHTS time — critical for the layer-2 GEMMs whose moving operand
is only E=8 columns. Accumulation stays fp32 in PSUM; epilogues are
split across the Scalar (ACT) and Vector (DVE) engines so neither
stalls the PE.
"""

import numpy as np
import ml_dtypes

B, F, H, C, E = 256, 2048, 512, 512, 8
K = H // 2  # 256
NCORES = 8
CPC = C // NCORES  # classes per core = 64
FC = F // 128      # 16 f-chunks
HC = H // 128      # 4 h-chunks
KC = K // 128      # 2 k-chunks
BATCH = 8          # classes per logits-PSUM batch
FSPLIT = (4, 4, 4, 4)  # f-chunks per shared-stage DMA group
NG = len(FSPLIT)
NWARM = 28         # HAM warm-up matmuls (sized to end as the first fw DMA lands)

BF16 = ml_dtypes.bfloat16

_PROGRAM = None


def _build_program():
    from contextlib import ExitStack

    import concourse.bass as bass
    import concourse.mybir as mybir
    import concourse.tile as tile
    from concourse import bacc

    f32 = mybir.dt.float32
    bf16 = mybir.dt.bfloat16
    Alu = mybir.AluOpType
    Act = mybir.ActivationFunctionType

    nc = bacc.Bacc(
        "TRN2", target_bir_lowering=False, debug=False, num_devices=NCORES
    )

    # fw: per f-chunk, featT[fc] (B cols) and ws[fc] (H cols) interleaved so
    # one DMA delivers matching moving+stationary data for a group of fcs.
    fw = nc.dram_tensor("fw", [128, FC, B + H], bf16, kind="ExternalInput").ap()
    w2 = nc.dram_tensor("w2", [128, CPC, KC, E], bf16, kind="ExternalInput").ap()
    bs = nc.dram_tensor("bs", [128, HC], f32, kind="ExternalInput").ap()
    w1 = nc.dram_tensor(
        "w1", [128, CPC, HC, KC, 128], bf16, kind="ExternalInput"
    ).ap()
    b1 = nc.dram_tensor("b1", [128, CPC, KC], f32, kind="ExternalInput").ap()
    b2 = nc.dram_tensor("b2", [128, CPC * E], f32, kind="ExternalInput").ap()
    out_lg = nc.dram_tensor(
        "out_logits", [B, CPC * E], bf16, kind="ExternalOutput"
    ).ap()
    out_gw = nc.dram_tensor(
        "out_gw", [B, CPC * E], bf16, kind="ExternalOutput"
    ).ap()

    with tile.TileContext(nc) as tc, ExitStack() as ctx:
        const = ctx.enter_context(tc.tile_pool(name="const", bufs=1))
        spool = ctx.enter_context(tc.tile_pool(name="sharedT", bufs=1))
        htpool0 = ctx.enter_context(tc.tile_pool(name="ht0", bufs=4))
        htpool1 = ctx.enter_context(tc.tile_pool(name="ht1", bufs=4))
        outpool = ctx.enter_context(tc.tile_pool(name="outs", bufs=1))

        # ---- constant loads, HWDGE, two rings ------------------------------
        # Sync ring (FIFO): fw groups (shared stage), then the 16 MiB W1 slab
        # in slices (first two halved so class 0's weights land right as the
        # shared stage finishes). Scalar ring: the small bias/W2 consts, so
        # they don't queue behind 3 MiB of fw traffic. Only 8 DMA
        # completion-sem lanes exist, so the head keeps at most 8 DMAs in
        # flight; later issues' lane-reuse waits are long-satisfied.
        fwg_sb = []
        f0 = 0
        bs_sb = const.tile([128, HC], f32)
        nc.scalar.dma_start(out=bs_sb[:], in_=bs[:])
        for g, gsz in enumerate(FSPLIT):
            t = const.tile([128, gsz, B + H], bf16, name=f"fwg{g}", tag=f"fwg{g}")
            nc.sync.dma_start(out=t[:], in_=fw[:, f0 : f0 + gsz, :])
            fwg_sb.append(t)
            f0 += gsz
        b1_sb = const.tile([128, CPC, KC], f32)
        nc.scalar.dma_start(out=b1_sb[:], in_=b1[:])
        w2_sb = const.tile([128, CPC, KC, E], bf16)
        nc.scalar.dma_start(out=w2_sb[:], in_=w2[:])
        b2_sb = const.tile([128, CPC * E], f32)
        nc.scalar.dma_start(out=b2_sb[:], in_=b2[:])
        # W1 resident in SBUF (64 classes x 2 KiB/partition = 128 KiB/part).
        w1_sb = const.tile([128, CPC, HC, KC, 128], bf16)
        w1_slices = [(0, 2), (2, 4), (4, 8)] + [
            (8 * g, 8 * g + 8) for g in range(1, 8)
        ]
        for c0, c1 in w1_slices:
            nc.sync.dma_start(out=w1_sb[:, c0:c1], in_=w1[:, c0:c1])

        # ---- shared transform: sharedT[h, b] = relu(Ws.T @ featT + bs) ------
        # fc-major loop with four persistent PSUM banks (one per h-chunk) so
        # compute on DMA group g overlaps the load of group g+1.
        sh_sb = spool.tile([128, HC, B], bf16)
        with tc.tile_pool(name="ps_sh", bufs=1, space="PSUM") as ps_sh:
            ps_list = [
                ps_sh.tile([128, B], f32, name=f"pssh{hc}", tag=f"pssh{hc}")
                for hc in range(HC)
            ]
            # HAM warm-up: keep the PE busy while the first const DMA lands
            # so the clock gate opens (1.2 -> 2.4 GHz) before real work.
            warm_sb = const.tile([128, B], bf16, name="warm_sb")
            nc.vector.memset(warm_sb[:], 0.0)
            # preload the ACT function tables off the critical path — the
            # first activation otherwise pays a ~1.3 us ACT_TABLE_LOAD right
            # when the shared-stage relu should run
            warm_act = const.tile([128, 1], f32, name="warm_act")
            nc.scalar.activation(
                out=warm_act[:], in_=warm_sb[:, 0:1], func=Act.Relu
            )
            nc.scalar.activation(
                out=warm_act[:], in_=warm_sb[:, 0:1], func=Act.Exp
            )
            warm_ps = ps_sh.tile([128, B], f32, name="warm_ps", tag="dummy_ps", bufs=1)
            for i in range(NWARM):
                nc.tensor.matmul(
                    warm_ps[:],
                    lhsT=warm_sb[:, :128],
                    rhs=warm_sb[:],
                    start=True,
                    stop=True,
                )

            def emit_shared_relu(hc):
                # relu+bias epilogues alternate ACT/DVE so they drain the
                # four PSUM banks in parallel right behind the final matmuls
                if hc % 2 == 0:
                    nc.scalar.activation(
                        out=sh_sb[:, hc, :],
                        in_=ps_list[hc][:],
                        func=Act.Relu,
                        bias=bs_sb[:, hc : hc + 1],
                    )
                else:
                    nc.vector.tensor_scalar(
                        out=sh_sb[:, hc, :],
                        in0=ps_list[hc][:],
                        scalar1=bs_sb[:, hc : hc + 1],
                        scalar2=0.0,
                        op0=Alu.add,
                        op1=Alu.max,
                    )

            for g, gsz in enumerate(FSPLIT):
                for fl in range(gsz):
                    first = g == 0 and fl == 0
                    last = g == NG - 1 and fl == gsz - 1
                    for hc in range(HC):
                        nc.tensor.matmul(
                            ps_list[hc][:],
                            lhsT=fwg_sb[g][:, fl, B + 128 * hc : B + 128 * (hc + 1)],
                            rhs=fwg_sb[g][:, fl, :B],
                            start=first,
                            stop=last,
                        )
                        if last:
                            emit_shared_relu(hc)


        ps_ht = ctx.enter_context(
            tc.tile_pool(name="ps_ht", bufs=4, space="PSUM")
        )
        ps_lg = ctx.enter_context(
            tc.tile_pool(name="ps_lg", bufs=2, space="PSUM")
        )

        # ---- output accumulation tiles (SBUF-resident) ----------------------
        lg_sb = [outpool.tile([128, CPC * E], bf16, name=f"lg{bc}", tag=f"lg{bc}") for bc in range(2)]
        gw_sb = [outpool.tile([128, CPC * E], bf16, name=f"gw{bc}", tag=f"gw{bc}") for bc in range(2)]
        sums_sb = [outpool.tile([128, CPC], f32, name=f"sm{bc}", tag=f"sm{bc}") for bc in range(2)]
        rsum_sb = [outpool.tile([128, CPC], f32, name=f"rs{bc}", tag=f"rs{bc}") for bc in range(2)]

        # ---- per-class grouped GEMMs ---------------------------------------
        ps_l_of = {}  # batch -> per-bc logits PSUM tiles

        def emit_l2(c, ht0, ht1):
            # layer 2: logits[b, e] = hT.T @ W2[c]  (accumulated over kc)
            b, ci = divmod(c, BATCH)
            hts = (ht0, ht1)
            for bc in range(2):
                for kc in range(KC):
                    nc.tensor.matmul(
                        ps_l_of[b][bc][:, ci * E : (ci + 1) * E],
                        lhsT=hts[kc][:, bc * 128 : (bc + 1) * 128],
                        rhs=w2_sb[:, c, kc, :],
                        start=(kc == 0),
                        stop=(kc == KC - 1),
                    )

        def emit_add(b, bc, h0=0, h1=BATCH):
            # lg = ps_l + b2 (also the PSUM -> SBUF copy, frees the PSUM tile)
            lo, hi = (b * BATCH + h0) * E, (b * BATCH + h1) * E
            nc.vector.tensor_add(
                out=lg_sb[bc][:, lo:hi],
                in0=ps_l_of[b][bc][:, h0 * E : h1 * E],
                in1=b2_sb[:, lo:hi],
            )

        def emit_softmax(b, bc, h0=0, h1=BATCH, tail=False):
            # exp + segmented row sums + normalize. Mid-run, the reduce and
            # normalize go to the otherwise-idle GPSIMD so the DVE (which
            # feeds the PE's layer-2 via the relu epilogues) never backs up;
            # on the tail they stay on DVE to avoid cross-engine hops.
            veng = nc.vector if tail else nc.gpsimd
            lo, hi = (b * BATCH + h0) * E, (b * BATCH + h1) * E
            cl, ch = b * BATCH + h0, b * BATCH + h1
            nc.scalar.activation(
                out=gw_sb[bc][:, lo:hi],
                in_=lg_sb[bc][:, lo:hi],
                func=Act.Exp,
            )
            nc.vector.tensor_reduce(
                out=sums_sb[bc][:, cl:ch],
                in_=gw_sb[bc][:, lo:hi].rearrange("p (c e) -> p c e", e=E),
                axis=mybir.AxisListType.X,
                op=Alu.add,
            )
            nc.vector.reciprocal(
                out=rsum_sb[bc][:, cl:ch], in_=sums_sb[bc][:, cl:ch]
            )
            rs = rsum_sb[bc][:, cl:ch]
            rs_bcast = bass.AP(
                tensor=rs.tensor, offset=rs.offset, ap=[*rs.ap, [0, E]]
            )
            veng.tensor_tensor(
                out=gw_sb[bc][:, lo:hi].rearrange("p (c e) -> p c e", e=E),
                in0=gw_sb[bc][:, lo:hi].rearrange("p (c e) -> p c e", e=E),
                in1=rs_bcast,
                op=Alu.mult,
            )

        def emit_flush(b):
            # flush outputs incrementally on the scalar HWDGE ring (the sync
            # ring still has W1 input traffic queued FIFO behind it); the
            # final piece is a single batch so the tail flush is short
            flush_at = {3: (0, 4), 5: (4, 6), 6: (6, 7), 7: (7, 8)}
            if b not in flush_at:
                return
            b0, b1r = flush_at[b]
            flo, fhi = b0 * BATCH * E, b1r * BATCH * E
            for bc in range(2):
                nc.scalar.dma_start(
                    out=out_lg[bc * 128 : (bc + 1) * 128, flo:fhi],
                    in_=lg_sb[bc][:, flo:fhi],
                )
                nc.scalar.dma_start(
                    out=out_gw[bc * 128 : (bc + 1) * 128, flo:fhi],
                    in_=gw_sb[bc][:, flo:fhi],
                )

        def batch_tasks(b):
            # one micro-task per class slot, so the epilogue never bursts
            # onto the DVE/ACT queues ahead of the relus the PE is waiting on
            return [
                lambda: emit_add(b, 0),
                lambda: emit_add(b, 1),
                lambda: emit_softmax(b, 0),
                lambda: emit_softmax(b, 1),
                lambda: emit_flush(b),
            ]

        pipelined = None  # (c, ht0, ht1) whose L2 is deferred one class
        pending = []      # deferred epilogue micro-tasks
        for c in range(CPC):
            if c % BATCH == 0:
                ps_l_of[c // BATCH] = [
                    ps_lg.tile([128, BATCH * E], f32, name=f"psl{bc}", tag=f"psl{bc}")
                    for bc in range(2)
                ]
            # layer 1: hT[k, b] = relu(W1[c].T @ sharedT + b1[c]), both
            # kc chunks accumulated in one full PSUM bank
            ht0 = htpool0.tile([128, B], bf16)
            ht1 = htpool1.tile([128, B], bf16)
            ph = ps_ht.tile([128, KC * B], f32)
            for kc in range(KC):
                for hc in range(HC):
                    nc.tensor.matmul(
                        ph[:, kc * B : (kc + 1) * B],
                        lhsT=w1_sb[:, c, hc, kc, :],
                        rhs=sh_sb[:, hc, :],
                        start=(hc == 0),
                        stop=(hc == HC - 1),
                    )
            # bias+relu epilogues split across ACT (kc=0) and DVE (kc=1);
            # separate ht tiles so the two writes have no ordering edge
            nc.scalar.activation(
                out=ht0[:],
                in_=ph[:, 0:B],
                func=Act.Relu,
                bias=b1_sb[:, c, 0:1],
            )
            nc.vector.tensor_scalar(
                out=ht1[:],
                in0=ph[:, B : 2 * B],
                scalar1=b1_sb[:, c, 1:2],
                scalar2=0.0,
                op0=Alu.add,
                op1=Alu.max,
            )
            # run the PREVIOUS class's layer 2 now: its relu finished long
            # ago, so the PE never waits on the epilogue mid-stream. The
            # pipeline carries across batch boundaries; each completed
            # batch's epilogue drains one micro-task per class slot. The
            # LAST batch is split in half-batches so most of its softmax
            # chain overlaps the final classes' matmuls instead of
            # serializing after the last L2.
            if pipelined is not None:
                emit_l2(*pipelined)
                pc = pipelined[0]
                if pc == CPC - BATCH // 2 - 1:
                    lb = CPC // BATCH - 1
                    pending.extend([
                        lambda: emit_add(lb, 0, 0, BATCH // 2),
                        lambda: emit_add(lb, 1, 0, BATCH // 2),
                        lambda: emit_softmax(lb, 0, 0, BATCH // 2, tail=True),
                        lambda: emit_softmax(lb, 1, 0, BATCH // 2, tail=True),
                    ])
                elif pc % BATCH == BATCH - 1 and pc != CPC - 1:
                    pending.extend(batch_tasks(pc // BATCH))
            if pending:
                pending.pop(0)()
            pipelined = (c, ht0, ht1)
        emit_l2(*pipelined)
        lb = CPC // BATCH - 1
        for t in pending:
            t()
        emit_add(lb, 0, BATCH // 2, BATCH)
        emit_add(lb, 1, BATCH // 2, BATCH)
        # logits for the final batch can flush as soon as the adds are done
        flo, fhi = lb * BATCH * E, CPC * E
        for bc in range(2):
            nc.scalar.dma_start(
                out=out_lg[bc * 128 : (bc + 1) * 128, flo:fhi],
                in_=lg_sb[bc][:, flo:fhi],
            )
        emit_softmax(lb, 0, BATCH // 2, BATCH, tail=True)
        emit_softmax(lb, 1, BATCH // 2, BATCH, tail=True)
        for bc in range(2):
            nc.scalar.dma_start(
                out=out_gw[bc * 128 : (bc + 1) * 128, flo:fhi],
                in_=gw_sb[bc][:, flo:fhi],
            )

    nc.compile()
    return nc


def get_program():
    global _PROGRAM
    if _PROGRAM is None:
        _PROGRAM = _build_program()
    return _PROGRAM


def make_in_maps(features, Ws, bs, W1, b1, W2, b2):
    """Host-side resharding of the full inputs into per-core device layouts."""
    f32 = np.float32
    features = np.ascontiguousarray(features, dtype=f32)
    Ws = np.ascontiguousarray(Ws, dtype=f32)
    bs = np.ascontiguousarray(bs, dtype=f32)
    W1 = np.ascontiguousarray(W1, dtype=f32)
    b1 = np.ascontiguousarray(b1, dtype=f32)
    W2 = np.ascontiguousarray(W2, dtype=f32)
    b2 = np.ascontiguousarray(b2, dtype=f32)

    featT_dev = features.T.reshape(FC, 128, B).transpose(1, 0, 2)  # [128,FC,B]
    ws_dev = Ws.reshape(FC, 128, H).transpose(1, 0, 2)             # [128,FC,H]
    fw_dev = np.ascontiguousarray(
        np.concatenate([featT_dev, ws_dev], axis=2).astype(BF16)   # [128,FC,B+H]
    )
    bs_dev = np.ascontiguousarray(bs.reshape(HC, 128).T)

    in_maps = []
    for i in range(NCORES):
        c0 = i * CPC
        w1_dev = np.ascontiguousarray(
            W1[c0 : c0 + CPC]
            .reshape(CPC, HC, 128, KC, 128)
            .transpose(2, 0, 1, 3, 4)
            .astype(BF16)
        )
        b1_dev = np.ascontiguousarray(
            b1[c0 : c0 + CPC].reshape(CPC, KC, 128).transpose(2, 0, 1)
        )
        w2_dev = np.ascontiguousarray(
            W2[c0 : c0 + CPC].reshape(CPC, KC, 128, E).transpose(2, 0, 1, 3)
            .astype(BF16)
        )
        b2_dev = np.ascontiguousarray(
            np.broadcast_to(b2[c0 : c0 + CPC].reshape(1, CPC * E), (128, CPC * E))
        )
        in_maps.append(
            {
                "fw": fw_dev,
                "w2": w2_dev,
                "bs": bs_dev,
                "w1": w1_dev,
                "b1": b1_dev,
                "b2": b2_dev,
            }
        )
    return in_maps


def assemble(results):
    """Gather per-core [B, CPC*E] slabs into full [B, C, E] outputs."""
    gate_logits = np.empty((B, C, E), dtype=np.float32)
    gate_weights = np.empty((B, C, E), dtype=np.float32)
    for i, r in enumerate(results):
        c0 = i * CPC
        gate_logits[:, c0 : c0 + CPC, :] = (
            r["out_logits"].astype(np.float32).reshape(B, CPC, E)
        )
        gate_weights[:, c0 : c0 + CPC, :] = (
            r["out_gw"].astype(np.float32).reshape(B, CPC, E)
        )
    return gate_weights, gate_logits


def kernel(**inputs):
    from concourse.bass_utils import run_bass_kernel_spmd

    nc = get_program()
    in_maps = make_in_maps(**inputs)
    res = run_bass_kernel_spmd(nc, in_maps, core_ids=list(range(NCORES)))
    return assemble(res.results)


# revision 30
# speedup vs baseline: 1.0219x; 1.0219x over previous
"""Trainium2 Bass kernel for nn_PerClassGating (moe_routing).

Computes, for inputs features[B,F], Ws[F,H], bs[H], W1[C,H,K], b1[C,K],
W2[C,K,E], b2[C,E] (B=256, F=2048, H=512, K=H/2=256, C=512, E=8):

    shared      = relu(features @ Ws + bs)                 # [B, H]
    h           = relu(einsum('bh,chk->bck', shared, W1) + b1)
    gate_logits = einsum('bck,cke->bce', h, W2) + b2       # [B, C, E]
    gate_weights = softmax(gate_logits, axis=-1)

Sharding: the class dim C is split across 8 NeuronCores (64 classes per
core); features and the shared transform are replicated. No collectives —
each core produces a disjoint [B, 64, E] slab of both outputs.

All matmul operands are bf16, cast on the HOST (tolerance is 2e-2;
bf16 keeps rel-err ~4e-3). This halves the dominant W1 HBM traffic
vs fp32/fp32r (16 MiB/core instead of 32), lets every load use the
fast HWDGE DMA path (no SWDGE cast), and enables the PE's automatic
Fast-Weight-Load (FWL) for 128-col bf16 stationaries, which halves
LDWEIGHTS time — critical for the layer-2 GEMMs whose moving operand
is only E=8 columns. Accumulation stays fp32 in PSUM; epilogues are
split across the Scalar (ACT) and Vector (DVE) engines so neither
stalls the PE.
"""

import numpy as np
import ml_dtypes

B, F, H, C, E = 256, 2048, 512, 512, 8
K = H // 2  # 256
NCORES = 8
CPC = C // NCORES  # classes per core = 64
FC = F // 128      # 16 f-chunks
HC = H // 128      # 4 h-chunks
KC = K // 128      # 2 k-chunks
BATCH = 8          # classes per logits-PSUM batch
FSPLIT = (4, 4, 4, 4)  # f-chunks per shared-stage DMA group
NG = len(FSPLIT)
NWARM = 28         # HAM warm-up matmuls (sized to end as the first fw DMA lands)

BF16 = ml_dtypes.bfloat16

_PROGRAM = None


def _build_program():
    from contextlib import ExitStack

    import concourse.bass as bass
    import concourse.mybir as mybir
    import concourse.tile as tile
    from concourse import bacc

    f32 = mybir.dt.float32
    bf16 = mybir.dt.bfloat16
    Alu = mybir.AluOpType
    Act = mybir.ActivationFunctionType

    nc = bacc.Bacc(
        "TRN2", target_bir_lowering=False, debug=False, num_devices=NCORES
    )

    # fw: per f-chunk, featT[fc] (B cols) and ws[fc] (H cols) interleaved so
    # one DMA delivers matching moving+stationary data for a group of fcs.
    fw = nc.dram_tensor("fw", [128, FC, B + H], bf16, kind="ExternalInput").ap()
    w2 = nc.dram_tensor("w2", [128, CPC, KC, E], bf16, kind="ExternalInput").ap()
    bs = nc.dram_tensor("bs", [128, HC], f32, kind="ExternalInput").ap()
    w1 = nc.dram_tensor(
        "w1", [128, CPC, HC, KC, 128], bf16, kind="ExternalInput"
    ).ap()
    b1 = nc.dram_tensor("b1", [128, CPC, KC], f32, kind="ExternalInput").ap()
    b2 = nc.dram_tensor("b2", [128, CPC * E], f32, kind="ExternalInput").ap()
    out_lg = nc.dram_tensor(
        "out_logits", [B, CPC * E], bf16, kind="ExternalOutput"
    ).ap()
    out_gw = nc.dram_tensor(
        "out_gw", [B, CPC * E], bf16, kind="ExternalOutput"
    ).ap()

    with tile.TileContext(nc) as tc, ExitStack() as ctx:
        const = ctx.enter_context(tc.tile_pool(name="const", bufs=1))
        spool = ctx.enter_context(tc.tile_pool(name="sharedT", bufs=1))
        htpool0 = ctx.enter_context(tc.tile_pool(name="ht0", bufs=4))
        htpool1 = ctx.enter_context(tc.tile_pool(name="ht1", bufs=4))
        outpool = ctx.enter_context(tc.tile_pool(name="outs", bufs=1))

        # ---- constant loads, HWDGE, two rings ------------------------------
        # Sync ring (FIFO): fw groups (shared stage), then the 16 MiB W1 slab
        # in slices (first two halved so class 0's weights land right as the
        # shared stage finishes). Scalar ring: the small bias/W2 consts, so
        # they don't queue behind 3 MiB of fw traffic. Only 8 DMA
        # completion-sem lanes exist, so the head keeps at most 8 DMAs in
        # flight; later issues' lane-reuse waits are long-satisfied.
        fwg_sb = []
        f0 = 0
        bs_sb = const.tile([128, HC], f32)
        nc.scalar.dma_start(out=bs_sb[:], in_=bs[:])
        for g, gsz in enumerate(FSPLIT):
            t = const.tile([128, gsz, B + H], bf16, name=f"fwg{g}", tag=f"fwg{g}")
            nc.sync.dma_start(out=t[:], in_=fw[:, f0 : f0 + gsz, :])
            fwg_sb.append(t)
            f0 += gsz
        b1_sb = const.tile([128, CPC, KC], f32)
        nc.scalar.dma_start(out=b1_sb[:], in_=b1[:])
        w2_sb = const.tile([128, CPC, KC, E], bf16)
        nc.scalar.dma_start(out=w2_sb[:], in_=w2[:])
        b2_sb = const.tile([128, CPC * E], f32)
        nc.scalar.dma_start(out=b2_sb[:], in_=b2[:])
        # W1 resident in SBUF (64 classes x 2 KiB/partition = 128 KiB/part).
        w1_sb = const.tile([128, CPC, HC, KC, 128], bf16)
        w1_slices = [(0, 2), (2, 4), (4, 8)] + [
            (8 * g, 8 * g + 8) for g in range(1, 8)
        ]
        for c0, c1 in w1_slices:
            nc.sync.dma_start(out=w1_sb[:, c0:c1], in_=w1[:, c0:c1])

        # ---- shared transform: sharedT[h, b] = relu(Ws.T @ featT + bs) ------
        # fc-major loop with four persistent PSUM banks (one per h-chunk) so
        # compute on DMA group g overlaps the load of group g+1.
        sh_sb = spool.tile([128, HC, B], bf16)
        with tc.tile_pool(name="ps_sh", bufs=1, space="PSUM") as ps_sh:
            ps_list = [
                ps_sh.tile([128, B], f32, name=f"pssh{hc}", tag=f"pssh{hc}")
                for hc in range(HC)
            ]
            # HAM warm-up: keep the PE busy while the first const DMA lands
            # so the clock gate opens (1.2 -> 2.4 GHz) before real work.
            warm_sb = const.tile([128, B], bf16, name="warm_sb")
            nc.vector.memset(warm_sb[:], 0.0)
            # preload the ACT function tables off the critical path — the
            # first activation otherwise pays a ~1.3 us ACT_TABLE_LOAD right
            # when the shared-stage relu should run
            warm_act = const.tile([128, 1], f32, name="warm_act")
            nc.scalar.activation(
                out=warm_act[:], in_=warm_sb[:, 0:1], func=Act.Relu
            )
            nc.scalar.activation(
                out=warm_act[:], in_=warm_sb[:, 0:1], func=Act.Exp
            )
            warm_ps = ps_sh.tile([128, B], f32, name="warm_ps", tag="dummy_ps", bufs=1)
            for i in range(NWARM):
                nc.tensor.matmul(
                    warm_ps[:],
                    lhsT=warm_sb[:, :128],
                    rhs=warm_sb[:],
                    start=True,
                    stop=True,
                )

            def emit_shared_relu(hc):
                # relu+bias epilogues alternate ACT/DVE so they drain the
                # four PSUM banks in parallel right behind the final matmuls
                if hc % 2 == 0:
                    nc.scalar.activation(
                        out=sh_sb[:, hc, :],
                        in_=ps_list[hc][:],
                        func=Act.Relu,
                        bias=bs_sb[:, hc : hc + 1],
                    )
                else:
                    nc.vector.tensor_scalar(
                        out=sh_sb[:, hc, :],
                        in0=ps_list[hc][:],
                        scalar1=bs_sb[:, hc : hc + 1],
                        scalar2=0.0,
                        op0=Alu.add,
                        op1=Alu.max,
                    )

            for g, gsz in enumerate(FSPLIT):
                for fl in range(gsz):
                    first = g == 0 and fl == 0
                    last = g == NG - 1 and fl == gsz - 1
                    for hc in range(HC):
                        nc.tensor.matmul(
                            ps_list[hc][:],
                            lhsT=fwg_sb[g][:, fl, B + 128 * hc : B + 128 * (hc + 1)],
                            rhs=fwg_sb[g][:, fl, :B],
                            start=first,
                            stop=last,
                        )
                        if last:
                            emit_shared_relu(hc)


        ps_ht = ctx.enter_context(
            tc.tile_pool(name="ps_ht", bufs=4, space="PSUM")
        )
        ps_lg = ctx.enter_context(
            tc.tile_pool(name="ps_lg", bufs=2, space="PSUM")
        )

        # ---- output accumulation tiles (SBUF-resident) ----------------------
        lg_sb = [outpool.tile([128, CPC * E], bf16, name=f"lg{bc}", tag=f"lg{bc}") for bc in range(2)]
        gw_sb = [outpool.tile([128, CPC * E], bf16, name=f"gw{bc}", tag=f"gw{bc}") for bc in range(2)]
        sums_sb = [outpool.tile([128, CPC], f32, name=f"sm{bc}", tag=f"sm{bc}") for bc in range(2)]
        rsum_sb = [outpool.tile([128, CPC], f32, name=f"rs{bc}", tag=f"rs{bc}") for bc in range(2)]

        # ---- per-class grouped GEMMs ---------------------------------------
        ps_l_of = {}  # batch -> per-bc logits PSUM tiles

        def emit_l2(c, ht0, ht1):
            # layer 2: logits[b, e] = hT.T @ W2[c]  (accumulated over kc)
            b, ci = divmod(c, BATCH)
            hts = (ht0, ht1)
            for bc in range(2):
                for kc in range(KC):
                    nc.tensor.matmul(
                        ps_l_of[b][bc][:, ci * E : (ci + 1) * E],
                        lhsT=hts[kc][:, bc * 128 : (bc + 1) * 128],
                        rhs=w2_sb[:, c, kc, :],
                        start=(kc == 0),
                        stop=(kc == KC - 1),
                    )

        def emit_add(b, bc, h0=0, h1=BATCH):
            # lg = ps_l + b2 (also the PSUM -> SBUF copy, frees the PSUM tile)
            lo, hi = (b * BATCH + h0) * E, (b * BATCH + h1) * E
            nc.vector.tensor_add(
                out=lg_sb[bc][:, lo:hi],
                in0=ps_l_of[b][bc][:, h0 * E : h1 * E],
                in1=b2_sb[:, lo:hi],
            )

        def emit_exp_red(b, bc, h0=0, h1=BATCH):
            # exp + segmented row sums
            lo, hi = (b * BATCH + h0) * E, (b * BATCH + h1) * E
            cl, ch = b * BATCH + h0, b * BATCH + h1
            nc.scalar.activation(
                out=gw_sb[bc][:, lo:hi],
                in_=lg_sb[bc][:, lo:hi],
                func=Act.Exp,
            )
            nc.vector.tensor_reduce(
                out=sums_sb[bc][:, cl:ch],
                in_=gw_sb[bc][:, lo:hi].rearrange("p (c e) -> p c e", e=E),
                axis=mybir.AxisListType.X,
                op=Alu.add,
            )

        def emit_norm(b, bc, h0=0, h1=BATCH, tail=False):
            # reciprocal + normalize; mid-run the multiply goes to the
            # otherwise-idle GPSIMD so the DVE (which feeds the PE's layer-2
            # via the relu epilogues) never backs up.
            veng = nc.vector if tail else nc.gpsimd
            lo, hi = (b * BATCH + h0) * E, (b * BATCH + h1) * E
            cl, ch = b * BATCH + h0, b * BATCH + h1
            nc.vector.reciprocal(
                out=rsum_sb[bc][:, cl:ch], in_=sums_sb[bc][:, cl:ch]
            )
            rs = rsum_sb[bc][:, cl:ch]
            rs_bcast = bass.AP(
                tensor=rs.tensor, offset=rs.offset, ap=[*rs.ap, [0, E]]
            )
            veng.tensor_tensor(
                out=gw_sb[bc][:, lo:hi].rearrange("p (c e) -> p c e", e=E),
                in0=gw_sb[bc][:, lo:hi].rearrange("p (c e) -> p c e", e=E),
                in1=rs_bcast,
                op=Alu.mult,
            )

        def emit_softmax(b, bc, h0=0, h1=BATCH, tail=False):
            emit_exp_red(b, bc, h0, h1)
            emit_norm(b, bc, h0, h1, tail=tail)

        def emit_flush(b):
            # flush outputs incrementally on the sync HWDGE ring (idle once
            # the W1 input traffic has drained; crucially NOT the scalar
            # queue, where a DMA issue would block the ACT engine's exp/relu
            # stream); the final piece is a single batch so the tail flush
            # is short
            flush_at = {3: (0, 4), 5: (4, 6), 6: (6, 7)}
            if b not in flush_at:
                return
            b0, b1r = flush_at[b]
            flo, fhi = b0 * BATCH * E, b1r * BATCH * E
            for bc in range(2):
                nc.sync.dma_start(
                    out=out_lg[bc * 128 : (bc + 1) * 128, flo:fhi],
                    in_=lg_sb[bc][:, flo:fhi],
                )
                nc.sync.dma_start(
                    out=out_gw[bc * 128 : (bc + 1) * 128, flo:fhi],
                    in_=gw_sb[bc][:, flo:fhi],
                )

        def batch_tasks(b):
            # one micro-task per class slot, so the epilogue never bursts
            # onto the DVE/ACT queues ahead of the relus the PE is waiting on
            return [
                lambda: emit_add(b, 0),
                lambda: emit_add(b, 1),
                lambda: emit_exp_red(b, 0),
                lambda: emit_norm(b, 0),
                lambda: emit_exp_red(b, 1),
                lambda: emit_norm(b, 1),
                lambda: emit_flush(b),
            ]

        pipelined = None  # (c, ht0, ht1) whose L2 is deferred one class
        pending = []      # deferred epilogue micro-tasks
        for c in range(CPC):
            if c % BATCH == 0:
                ps_l_of[c // BATCH] = [
                    ps_lg.tile([128, BATCH * E], f32, name=f"psl{bc}", tag=f"psl{bc}")
                    for bc in range(2)
                ]
            # layer 1: hT[k, b] = relu(W1[c].T @ sharedT + b1[c]), both
            # kc chunks accumulated in one full PSUM bank
            ht0 = htpool0.tile([128, B], bf16)
            ht1 = htpool1.tile([128, B], bf16)
            ph = ps_ht.tile([128, KC * B], f32)
            for kc in range(KC):
                for hc in range(HC):
                    nc.tensor.matmul(
                        ph[:, kc * B : (kc + 1) * B],
                        lhsT=w1_sb[:, c, hc, kc, :],
                        rhs=sh_sb[:, hc, :],
                        start=(hc == 0),
                        stop=(hc == HC - 1),
                    )
            # bias+relu epilogues split across ACT (kc=0) and DVE (kc=1);
            # separate ht tiles so the two writes have no ordering edge
            nc.scalar.activation(
                out=ht0[:],
                in_=ph[:, 0:B],
                func=Act.Relu,
                bias=b1_sb[:, c, 0:1],
            )
            nc.vector.tensor_scalar(
                out=ht1[:],
                in0=ph[:, B : 2 * B],
                scalar1=b1_sb[:, c, 1:2],
                scalar2=0.0,
                op0=Alu.add,
                op1=Alu.max,
            )
            # run the PREVIOUS class's layer 2 now: its relu finished long
            # ago, so the PE never waits on the epilogue mid-stream. The
            # pipeline carries across batch boundaries; each completed
            # batch's epilogue drains one micro-task per class slot. The
            # LAST batch is handled half-batch-wise with explicit placement
            # so most of its softmax chain overlaps the final classes'
            # matmuls instead of serializing after the last L2.
            lb = CPC // BATCH - 1
            if pipelined is not None:
                emit_l2(*pipelined)
                pc = pipelined[0]
                if pc == CPC - BATCH // 2 - 1:
                    emit_add(lb, 0, 0, BATCH // 2)
                    emit_add(lb, 1, 0, BATCH // 2)
                elif pc == CPC - BATCH // 2:
                    emit_softmax(lb, 0, 0, BATCH // 2, tail=True)
                elif pc == CPC - BATCH // 2 + 1:
                    emit_softmax(lb, 1, 0, BATCH // 2, tail=True)
                elif pc % BATCH == BATCH - 1 and pc != CPC - 1:
                    pending.extend(batch_tasks(pc // BATCH))
            if pending:
                pending.pop(0)()
            pipelined = (c, ht0, ht1)
        emit_l2(*pipelined)
        for t in pending:
            t()
        emit_add(lb, 0, BATCH // 2, BATCH)
        emit_add(lb, 1, BATCH // 2, BATCH)
        # logits for the final batch can flush as soon as the adds are done
        flo, fhi = lb * BATCH * E, CPC * E
        for bc in range(2):
            nc.sync.dma_start(
                out=out_lg[bc * 128 : (bc + 1) * 128, flo:fhi],
                in_=lg_sb[bc][:, flo:fhi],
            )
        emit_softmax(lb, 0, BATCH // 2, BATCH, tail=True)
        emit_softmax(lb, 1, BATCH // 2, BATCH, tail=True)
        for bc in range(2):
            nc.sync.dma_start(
                out=out_gw[bc * 128 : (bc + 1) * 128, flo:fhi],
                in_=gw_sb[bc][:, flo:fhi],
            )

    nc.compile()
    return nc


def get_program():
    global _PROGRAM
    if _PROGRAM is None:
        _PROGRAM = _build_program()
    return _PROGRAM


def make_in_maps(features, Ws, bs, W1, b1, W2, b2):
    """Host-side resharding of the full inputs into per-core device layouts."""
    f32 = np.float32
    features = np.ascontiguousarray(features, dtype=f32)
    Ws = np.ascontiguousarray(Ws, dtype=f32)
    bs = np.ascontiguousarray(bs, dtype=f32)
    W1 = np.ascontiguousarray(W1, dtype=f32)
    b1 = np.ascontiguousarray(b1, dtype=f32)
    W2 = np.ascontiguousarray(W2, dtype=f32)
    b2 = np.ascontiguousarray(b2, dtype=f32)

    featT_dev = features.T.reshape(FC, 128, B).transpose(1, 0, 2)  # [128,FC,B]
    ws_dev = Ws.reshape(FC, 128, H).transpose(1, 0, 2)             # [128,FC,H]
    fw_dev = np.ascontiguousarray(
        np.concatenate([featT_dev, ws_dev], axis=2).astype(BF16)   # [128,FC,B+H]
    )
    bs_dev = np.ascontiguousarray(bs.reshape(HC, 128).T)

    in_maps = []
    for i in range(NCORES):
        c0 = i * CPC
        w1_dev = np.ascontiguousarray(
            W1[c0 : c0 + CPC]
            .reshape(CPC, HC, 128, KC, 128)
            .transpose(2, 0, 1, 3, 4)
            .astype(BF16)
        )
        b1_dev = np.ascontiguousarray(
            b1[c0 : c0 + CPC].reshape(CPC, KC, 128).transpose(2, 0, 1)
        )
        w2_dev = np.ascontiguousarray(
            W2[c0 : c0 + CPC].reshape(CPC, KC, 128, E).transpose(2, 0, 1, 3)
            .astype(BF16)
        )
        b2_dev = np.ascontiguousarray(
            np.broadcast_to(b2[c0 : c0 + CPC].reshape(1, CPC * E), (128, CPC * E))
        )
        in_maps.append(
            {
                "fw": fw_dev,
                "w2": w2_dev,
                "bs": bs_dev,
                "w1": w1_dev,
                "b1": b1_dev,
                "b2": b2_dev,
            }
        )
    return in_maps


def assemble(results):
    """Gather per-core [B, CPC*E] slabs into full [B, C, E] outputs."""
    gate_logits = np.empty((B, C, E), dtype=np.float32)
    gate_weights = np.empty((B, C, E), dtype=np.float32)
    for i, r in enumerate(results):
        c0 = i * CPC
        gate_logits[:, c0 : c0 + CPC, :] = (
            r["out_logits"].astype(np.float32).reshape(B, CPC, E)
        )
        gate_weights[:, c0 : c0 + CPC, :] = (
            r["out_gw"].astype(np.float32).reshape(B, CPC, E)
        )
    return gate_weights, gate_logits


def kernel(**inputs):
    from concourse.bass_utils import run_bass_kernel_spmd

    nc = get_program()
    in_maps = make_in_maps(**inputs)
    res = run_bass_kernel_spmd(nc, in_maps, core_ids=list(range(NCORES)))
    return assemble(res.results)
